# revision 3
# baseline (speedup 1.0000x reference)
"""Causal multi-head attention on 8 TRN2 NeuronCores — v2 (ACT-optimized).

Problem: x[4, 2048, 768], 12 heads x d_head 64, causal softmax attention.

Sharding: core c handles batch b = c//2 and the 6-head group h0 = 6*(c%2).
Each core computes its partial output o_partial[2048, 768] = sum over its 6
heads of (softmax(QK^T/8) V) @ W_O.  The two cores sharing a batch are summed
on the host (part of unsharding), so the device graph needs no collectives.

v2 changes vs baseline (which was scalar-engine bound in the attention
phase: 168 ACTIVATEs, avg 945ns, ~160us of exp):
  - Attention processes one q-supertile (t) at a time per head pair, with a
    PSUM layout of pssA [128,2048] (4 banks) + pssB [128,1024] (2 banks)
    ping-pong for scores and psz [65,1024] (2 banks, both heads) for AV
    accumulation = exactly 8 banks.
  - Off-diagonal k-blocks are exp'd in PAIRS via one 2048-wide ACTIVATE
    (amortizes the ~350-cycle per-instruction overhead); diagonal blocks are
    exp'd solo with exact-width 3D APs.  ~87 ACTIVATEs instead of 168.
  - Projection drains are 2048-wide DVE casts (6 instead of 24).
  - psz for both heads is a single [65, 1024] tile -> one drain per t.
  - The reciprocal chain (DRAM bounce) and final ZT normalize muls are
    deferred by one t so their DMA latency never head-blocks the DVE queue.
  - Output projection reuses the attention PSUM region and overlaps the
    final normalize tail.
"""

import sys

if "/opt/trn_rl_repo" not in sys.path:
    sys.path.insert(0, "/opt/trn_rl_repo")

import numpy as np
import ml_dtypes

BF16NP = ml_dtypes.bfloat16


def _ensure_ntff_hook():
    """The agent image's `antenv` lacks `axon_hooks`, which bass_utils needs
    for trace=True under axon. Recreate it via sys.modules injection using the
    boot helper's ctypes wrapper around libaxon_pjrt.so."""
    import types
    if "antenv.axon_hooks" in sys.modules:
        return
    try:
        from trn_agent_boot.trn_boot import _ntff_profile_via_ctypes
        hook = _ntff_profile_via_ctypes("/opt/axon/libaxon_pjrt.so")
    except Exception:
        hook = None
    m = types.ModuleType("antenv.axon_hooks")
    m._hook = hook
    m.get_axon_ntff_profile_hook = lambda: m._hook
    def _set(h):
        m._hook = h
    m.set_axon_ntff_profile_hook = _set
    sys.modules["antenv.axon_hooks"] = m


_ensure_ntff_hook()

import concourse.bass as bass
import concourse.tile as tile
from concourse import bacc, mybir
from concourse.bass_utils import run_bass_kernel_spmd

F32 = mybir.dt.float32
BF16 = mybir.dt.bfloat16
AF = mybir.ActivationFunctionType

D = 768          # d_model
S = 2048         # seq
E = 64           # d_head
NHC = 6          # heads per core
HE = NHC * E     # 384
KD = D // 128    # 6 k-chunks over d_model
B = 4

LAST_EXEC_TIME_NS = None
_GRAPH_CACHE = {}


def _slots_for_t(t):
    """Slot schedule for q-supertile t: alternating A (2048-wide psum) and
    B (1024-wide psum) slots.  A prefers pairs of off-diagonal k-blocks,
    B takes diagonal blocks solo (exact-width exp)."""
    offs = list(range(4 * t))
    diags = list(range(4 * t, 4 * t + 4))
    slots = []
    use_a = True
    while offs or diags:
        if use_a:
            if len(offs) >= 2:
                js = [offs.pop(0), offs.pop(0)]
            elif offs:
                js = [offs.pop(0)]
            else:
                js = [diags.pop(0)]
            slots.append(("A", js))
        else:
            if diags:
                js = [diags.pop(0)]
            else:
                js = [offs.pop(0)]
            slots.append(("B", js))
        use_a = not use_a
    return slots


def _build_graph(qkv_bias: bool) -> bass.Bass:
    nc = bacc.Bacc("TRN2", target_bir_lowering=False)
    xt = nc.declare_dram_parameter("xt", [D, S], BF16, isOutput=False)
    wq = nc.declare_dram_parameter("wq", [D, HE], BF16, isOutput=False)
    wk = nc.declare_dram_parameter("wk", [D, HE], BF16, isOutput=False)
    wv = nc.declare_dram_parameter("wv", [D, HE], BF16, isOutput=False)
    wo = nc.declare_dram_parameter("wo", [HE, D], BF16, isOutput=False)
    mask = nc.declare_dram_parameter("mask", [128, 128], BF16, isOutput=False)
    if qkv_bias:
        bq = nc.declare_dram_parameter("bq", [HE, 1], F32, isOutput=False)
        bk = nc.declare_dram_parameter("bk", [HE, 1], F32, isOutput=False)
        bv = nc.declare_dram_parameter("bv", [1, HE], BF16, isOutput=False)
    out = nc.declare_dram_parameter("out", [S, D], BF16, isOutput=True)

    with tile.TileContext(nc) as tc:
        with tc.tile_pool(name="persist", bufs=1) as persist:
            QT = [persist.tile([128, S], BF16, tag=f"qt{m}", name=f"qt{m}") for m in range(3)]
            KT = [persist.tile([128, S], BF16, tag=f"kt{m}", name=f"kt{m}") for m in range(3)]
            ZT = [persist.tile([128, S], BF16, tag=f"zt{m}", name=f"zt{m}") for m in range(3)]
            VA = [persist.tile([128, NHC * 65], BF16, tag=f"va{s}", name=f"va{s}") for s in range(16)]
            WO = [persist.tile([128, D], BF16, tag=f"wo{m}", name=f"wo{m}") for m in range(3)]
            MSK = persist.tile([128, 128], BF16, tag="mask", name="mask_sb")
            # preload the exp table set ASAP (2.7us, overlaps the lead-in DMAs)
            warm = persist.tile([1, 8], BF16, tag="warm", name="warm")
            nc.vector.memset(warm[:], 0.0)
            nc.scalar.activation(warm[:], warm[:], AF.Exp)
            ONES = persist.tile([1, 128], BF16, tag="ones", name="ones_sb")
            nc.vector.memset(ONES[:], 1.0)
            if qkv_bias:
                BQ = persist.tile([128, 3], F32, tag="bq", name="bq_sb")
                BK = persist.tile([128, 3], F32, tag="bk", name="bk_sb")
                BV = persist.tile([1, HE], BF16, tag="bv", name="bv_sb")
                for m in range(3):
                    nc.sync.dma_start(out=BQ[:, m:m + 1], in_=bq[m * 128:(m + 1) * 128, :])
                    nc.sync.dma_start(out=BK[:, m:m + 1], in_=bk[m * 128:(m + 1) * 128, :])
                nc.sync.dma_start(out=BV[:], in_=bv[:])

            # ---------------- projections ----------------
            with tc.tile_pool(name="loadA", bufs=1) as loadA, \
                 tc.tile_pool(name="psP", bufs=2, space="PSUM") as psP:
                XT = [loadA.tile([128, S], BF16, tag=f"xt{k}", name=f"xt{k}") for k in range(KD)]
                WQs = [loadA.tile([128, HE], BF16, tag=f"wq{k}", name=f"wq{k}") for k in range(KD)]
                WKs = [loadA.tile([128, HE], BF16, tag=f"wk{k}", name=f"wk{k}") for k in range(KD)]
                WVs = [loadA.tile([128, HE], BF16, tag=f"wv{k}", name=f"wv{k}") for k in range(KD)]
                # two hardware DMA queues (sync + scalar) halve the input
                # stream time; chunks interleaved in consumption order
                for k in range(KD):
                    if k % 2 == 0:
                        nc.scalar.dma_start(out=XT[k][:], in_=xt[k * 128:(k + 1) * 128, :])
                    else:
                        nc.sync.dma_start(out=XT[k][:], in_=xt[k * 128:(k + 1) * 128, :])
                    nc.scalar.dma_start(out=WQs[k][:], in_=wq[k * 128:(k + 1) * 128, :])
                    nc.sync.dma_start(out=WKs[k][:], in_=wk[k * 128:(k + 1) * 128, :])
                for k in range(KD):
                    nc.sync.dma_start(out=WVs[k][:], in_=wv[k * 128:(k + 1) * 128, :])
                for m in range(3):
                    nc.scalar.dma_start(out=WO[m][:], in_=wo[m * 128:(m + 1) * 128, :])
                nc.scalar.dma_start(out=MSK[:], in_=mask[:])

                # Q^T and K^T: [384, 2048] = W.T @ x^T, one [128, 2048] psum
                # fill (4 banks) per m-block, drained by a single 2048-wide
                # cast.  k-outer / n-inner keeps 4 matmuls runnable per
                # arriving xt chunk so the PE stays dense (and HAM warm)
                # during the input DMA ramp.
                for m in range(3):
                    for Wt, Ot, Bt in ((WQs, QT, "bq"), (WKs, KT, "bk")):
                        ps = psP.tile([128, 2048], F32, tag="psP", name="ps_proj")
                        for k in range(KD):
                            for n in range(4):
                                nc.tensor.matmul(
                                    ps[:, n * 512:(n + 1) * 512],
                                    Wt[k][:, m * 128:(m + 1) * 128],
                                    XT[k][:, n * 512:(n + 1) * 512],
                                    start=(k == 0), stop=(k == KD - 1))
                        if qkv_bias:
                            bias_t = BQ if Bt == "bq" else BK
                            nc.scalar.activation(Ot[m][:], ps[:], AF.Identity,
                                                 bias=bias_t[:, m:m + 1])
                        else:
                            nc.vector.tensor_copy(Ot[m][:], ps[:])

                # V (augmented with ones column per head): VA[sc] = [128, 6*65]
                # four sc-chunks share one [128, 2048] psum fill (512-aligned)
                for g in range(4):
                    psv = psP.tile([128, 2048], F32, tag="psP", name="ps_v")
                    for si in range(4):
                        sc = 4 * g + si
                        nc.vector.memset(VA[sc][:], 1.0)
                        for k in range(KD):
                            nc.tensor.matmul(
                                psv[:, si * 512:si * 512 + HE],
                                XT[k][:, sc * 128:(sc + 1) * 128],
                                WVs[k][:],
                                start=(k == 0),
                                stop=False if qkv_bias else (k == KD - 1))
                        if qkv_bias:
                            nc.tensor.matmul(
                                psv[:, si * 512:si * 512 + HE], ONES[:], BV[:],
                                start=False, stop=True)
                    for si in range(4):
                        sc = 4 * g + si
                        nc.vector.tensor_copy(
                            VA[sc][:].rearrange("p (h c) -> p h c", c=65)[:, :, 0:64],
                            psv[:, si * 512:si * 512 + HE].rearrange(
                                "p (h c) -> p h c", c=64))

            # ---------------- attention ----------------
            with tc.tile_pool(name="workE", bufs=3) as workE, \
                 tc.tile_pool(name="workZ", bufs=3) as workZ, \
                 tc.tile_pool(name="work2", bufs=2) as work2, \
                 tc.tile_pool(name="workO", bufs=3) as workO, \
                 tc.tile_pool(name="dramP", bufs=2, space="DRAM") as dramP, \
                 tc.tile_pool(name="psATT", bufs=1, space="PSUM") as psATT:
                deferred_a = []  # per-t [recip + rcd/bc DMA] closures, 1-t lag
                deferred_b = []  # per-t [ZT muls] closures, 2-t lag (bc landed)

                def emit_normalize(hp, t, psz):
                    # first stage (drain + denominator row to DRAM) now;
                    # reciprocal + ZT muls deferred so the DMA latency never
                    # head-blocks the vector queue
                    zraw = workZ.tile([65, 1024], BF16, tag="zraw", name="zraw")
                    nc.vector.tensor_copy(zraw[:], psz[:])
                    dd = dramP.tile([1, 1024], BF16, tag="dd", name="dd")
                    nc.sync.dma_start(out=dd[:], in_=zraw[64:65, :])
                    dd_ap = dd[:]
                    rp = work2.tile([128, 8], BF16, tag="rp", name="rp")
                    nc.sync.dma_start(out=rp[:], in_=bass.AP(
                        tensor=dd_ap.tensor, offset=dd_ap.offset,
                        ap=[[8, 128], [1, 8]]))

                    st = {}

                    def finish_a(rp=rp, st=st):
                        rcp = work2.tile([128, 8], BF16, tag="rcp", name="rcp")
                        with nc.allow_low_precision(reason="softmax recip bf16"):
                            nc.vector.reciprocal(rcp[:], rp[:])
                        rcd = dramP.tile([1, 1024], BF16, tag="rcd", name="rcd")
                        rcd_ap = rcd[:]
                        nc.sync.dma_start(out=bass.AP(
                            tensor=rcd_ap.tensor, offset=rcd_ap.offset,
                            ap=[[8, 128], [1, 8]]), in_=rcp[:])
                        bc = work2.tile([64, 1024], BF16, tag="bc", name="bc")
                        nc.sync.dma_start(out=bc[:], in_=bass.AP(
                            tensor=rcd_ap.tensor, offset=rcd_ap.offset,
                            ap=[[0, 64], [1, 1024]]))
                        st["bc"] = bc

                    def finish_b(hp=hp, t=t, zraw=zraw, st=st):
                        bc = st["bc"]
                        for par in (0, 1):
                            nc.vector.tensor_mul(
                                ZT[hp][par * 64:par * 64 + 64,
                                       t * 512:(t + 1) * 512],
                                zraw[0:64, par * 512:(par + 1) * 512],
                                bc[:, par * 512:(par + 1) * 512])
                    deferred_a.append(finish_a)
                    deferred_b.append(finish_b)
                    if len(deferred_a) > 1:
                        deferred_a.pop(0)()
                    if len(deferred_b) > 2:
                        deferred_b.pop(0)()

                # Global group schedule: each group = up to 2 slots (one A
                # [128,2048] + one B [128,1024]).  Per group we emit scores,
                # then both ACTIVATEs back-to-back, then masks, then the AV
                # matmuls of the PREVIOUS group — so the PE has the whole
                # combined ACT window's worth of work (prev 3 AVs + next
                # group's scores trickling in as its buffers free up).
                groups = []
                for hp in range(3):
                    for t in range(4):
                        slots = _slots_for_t(t)
                        fl = (slots[0][1][0], slots[-1][1][-1])
                        gs = [slots[i:i + 2] for i in range(0, len(slots), 2)]
                        for gi, g in enumerate(gs):
                            groups.append(dict(
                                hp=hp, t=t, slots=g, fl=fl,
                                t_last=(gi == len(gs) - 1)))

                psz_by = {}
                pending = None  # (av closures, group meta) of previous group
                for g in groups:
                    hp, t = g["hp"], g["t"]
                    if (hp, t) not in psz_by:
                        psz_by[(hp, t)] = psATT.tile([65, 1024], F32,
                                                     tag="psz", name="psz")
                    psz = psz_by[(hp, t)]
                    first_j, last_j = g["fl"]
                    slot_work = []  # (buf, js, pss, et)
                    # scores for every slot in the group
                    for buf, js in g["slots"]:
                        width = 2048 if buf == "A" else 1024
                        pss = psATT.tile([128, width], F32,
                                         tag=f"pss{buf}", name=f"pss{buf}")
                        et = workE.tile([128, width], BF16,
                                        tag=f"et{buf}", name=f"et{buf}")
                        for half, j in enumerate(js):
                            r = j - 4 * t
                            q0 = 128 * r if r >= 0 else 0
                            for par in (0, 1):
                                ho = par * 64
                                cb = half * 1024 + par * 512
                                nc.tensor.matmul(
                                    pss[:, cb + q0:cb + 512],
                                    KT[hp][ho:ho + 64, j * 128:(j + 1) * 128],
                                    QT[hp][ho:ho + 64,
                                           t * 512 + q0:(t + 1) * 512],
                                    start=True, stop=True)
                        slot_work.append((buf, js, pss, et))
                    # both ACTIVATEs back-to-back
                    for buf, js, pss, et in slot_work:
                        r0 = js[0] - 4 * t
                        if len(js) == 2:
                            nc.scalar.activation(et[:], pss[:], AF.Exp,
                                                 scale=0.125)
                        elif r0 < 0:
                            nc.scalar.activation(et[:, 0:1024], pss[:, 0:1024],
                                                 AF.Exp, scale=0.125)
                        else:
                            q0 = 128 * r0
                            pss_ap = pss[:]
                            et_ap = et[:]
                            w = 512 - q0
                            in3 = bass.AP(
                                tensor=pss_ap.tensor,
                                offset=pss_ap.offset + q0,
                                ap=[pss_ap.ap[0], [512, 2], [1, w]])
                            out3 = bass.AP(
                                tensor=et_ap.tensor,
                                offset=et_ap.offset + q0,
                                ap=[et_ap.ap[0], [512, 2], [1, w]])
                            nc.scalar.activation(out3, in3, AF.Exp, scale=0.125)
                    # masks on diagonal 128-blocks (post-exp 0/1 multiply)
                    for buf, js, pss, et in slot_work:
                        for half, j in enumerate(js):
                            r = j - 4 * t
                            if r >= 0:
                                q0 = 128 * r
                                for par in (0, 1):
                                    cb = half * 1024 + par * 512
                                    nc.vector.tensor_mul(
                                        et[:, cb + q0:cb + q0 + 128],
                                        et[:, cb + q0:cb + q0 + 128],
                                        MSK[:])
                    # AV matmuls of the previous group
                    if pending is not None:
                        avs, meta = pending
                        for av in avs:
                            av()
                        if meta["t_last"]:
                            emit_normalize(meta["hp"], meta["t"],
                                           psz_by[(meta["hp"], meta["t"])])
                    avs = []
                    for buf, js, pss, et in slot_work:
                        for half, j in enumerate(js):
                            r = j - 4 * t
                            q0 = 128 * r if r >= 0 else 0
                            def av_fn(j=j, q0=q0, et=et, half=half, hp=hp,
                                      psz=psz, first_j=first_j, last_j=last_j):
                                for par in (0, 1):
                                    h = 2 * hp + par
                                    cb = half * 1024 + par * 512
                                    nc.tensor.matmul(
                                        psz[:, par * 512 + q0:par * 512 + 512],
                                        VA[j][:, h * 65:(h + 1) * 65],
                                        et[:, cb + q0:cb + 512],
                                        start=(j == first_j),
                                        stop=(j == last_j))
                            avs.append(av_fn)
                    pending = (avs, g)
                avs, meta = pending
                for av in avs:
                    av()
                emit_normalize(meta["hp"], meta["t"],
                               psz_by[(meta["hp"], meta["t"])])
                while deferred_a:
                    deferred_a.pop(0)()
                while deferred_b:
                    deferred_b.pop(0)()

                # ---------------- output projection ----------------
                # reuses the attention psum regions (pssA for even chunks,
                # pssB for odd) as [128, 384]-pair accumulators at
                # bank-aligned offsets, so chunks ping-pong across banks
                for mc in range(16):
                    if mc % 2 == 0:
                        po = psATT.tile([128, 2048], F32, tag="pssA", name="poA")
                        hoff = 1024
                    else:
                        po = psATT.tile([128, 1024], F32, tag="pssB", name="poB")
                        hoff = 512
                    for half in (0, 1):
                        n0 = half * HE
                        for k in range(3):
                            nc.tensor.matmul(
                                po[:, half * hoff:half * hoff + HE],
                                ZT[k][:, mc * 128:(mc + 1) * 128],
                                WO[k][:, n0:n0 + HE],
                                start=(k == 0), stop=(k == 2))
                    ob = workO.tile([128, D], BF16, tag="ob", name="ob")
                    po_ap = po[:]
                    ob_ap = ob[:]
                    nc.vector.tensor_copy(
                        bass.AP(tensor=ob_ap.tensor, offset=ob_ap.offset,
                                ap=[ob_ap.ap[0], [HE, 2], [1, HE]]),
                        bass.AP(tensor=po_ap.tensor, offset=po_ap.offset,
                                ap=[po_ap.ap[0], [hoff, 2], [1, HE]]))
                    dma_eng = nc.sync if mc % 2 == 0 else nc.scalar
                    dma_eng.dma_start(
                        out=out[mc * 128:(mc + 1) * 128, :], in_=ob[:])
    nc.compile()
    return nc


def _build_mask() -> np.ndarray:
    # triangle for the strict-diagonal 128x128 strip: 1.0 iff q_local >= k_local
    kl = np.arange(128)[:, None]
    ql = np.arange(128)[None, :]
    return (ql >= kl).astype(np.float32)


def kernel(**inputs) -> np.ndarray:
    global LAST_EXEC_TIME_NS
    x = np.asarray(inputs["normalized_resid_pre"], dtype=np.float32)
    W_Q = np.asarray(inputs["W_Q"], dtype=np.float32)
    W_K = np.asarray(inputs["W_K"], dtype=np.float32)
    W_V = np.asarray(inputs["W_V"], dtype=np.float32)
    W_O = np.asarray(inputs["W_O"], dtype=np.float32)
    b_Q = np.asarray(inputs["b_Q"], dtype=np.float32)
    b_K = np.asarray(inputs["b_K"], dtype=np.float32)
    b_V = np.asarray(inputs["b_V"], dtype=np.float32)
    b_O = np.asarray(inputs["b_O"], dtype=np.float32)

    qkv_bias = bool(b_Q.any() or b_K.any() or b_V.any())
    key = qkv_bias
    if key not in _GRAPH_CACHE:
        _GRAPH_CACHE[key] = _build_graph(qkv_bias)
    nc = _GRAPH_CACHE[key]

    mask = _build_mask()
    in_maps = []
    for c in range(8):
        b, h0 = c // 2, NHC * (c % 2)
        im = {
            "xt": np.ascontiguousarray(x[b].T).astype(BF16NP),
            "wq": np.ascontiguousarray(
                W_Q[h0:h0 + NHC].transpose(1, 0, 2).reshape(D, HE)).astype(BF16NP),
            "wk": np.ascontiguousarray(
                W_K[h0:h0 + NHC].transpose(1, 0, 2).reshape(D, HE)).astype(BF16NP),
            "wv": np.ascontiguousarray(
                W_V[h0:h0 + NHC].transpose(1, 0, 2).reshape(D, HE)).astype(BF16NP),
            "wo": np.ascontiguousarray(W_O[h0:h0 + NHC].reshape(HE, D)).astype(BF16NP),
            "mask": mask.astype(BF16NP),
        }
        if qkv_bias:
            im["bq"] = np.ascontiguousarray(b_Q[h0:h0 + NHC].reshape(HE, 1))
            im["bk"] = np.ascontiguousarray(b_K[h0:h0 + NHC].reshape(HE, 1))
            im["bv"] = np.ascontiguousarray(b_V[h0:h0 + NHC].reshape(1, HE)).astype(BF16NP)
        in_maps.append(im)

    import os
    trace = bool(os.environ.get("KERNEL_TRACE"))
    res = run_bass_kernel_spmd(nc, in_maps, core_ids=list(range(8)), trace=trace)
    LAST_EXEC_TIME_NS = res.exec_time_ns
    results = res.results

    out = np.empty((B, S, D), dtype=np.float32)
    for b in range(B):
        out[b] = (results[2 * b]["out"].astype(np.float32)
                  + results[2 * b + 1]["out"].astype(np.float32))
    if b_O.any():
        out += b_O
    return out
